# revision 35
# baseline (speedup 1.0000x reference)
"""Trainium2 Bass kernel for CNCAgg (weight-MLP + bmm aggregation + final 1x1 conv).

Strategy: data-parallel over batch B=32 across 8 NeuronCores (4 batches/core).
All BN layers are folded into the conv weights on the host (eval-mode BN is an
affine transform). Per core:
  1. WeightNet 3->64->64->64 (conv1x1+BN+ReLU x3) on the PE array, batches
     pair-stacked along the PSUM free axis for full-width activations.
  2. Layer 3 emits its output transposed (n on partitions) so the bmm
     agg^T[b] = sum_n wgt[n,:]^T feat[n,:] accumulates in PSUM with no
     on-chip transpose of `feature` (host pre-transposes feature to (B,N,C)).
  3. agg is shuffled in-SBUF into (cw, k, b) layout; the final conv runs as
     out^T[b,o] = sum_k agg_s[:,k,:].T @ wfT[k] with wf' as the wide moving
     operand (N=512) so fp32->bf16 weight streaming hits full PE rate.
"""

import os
import sys

sys.path.insert(0, "/opt/trn_rl_repo")

import numpy as np
import ml_dtypes  # noqa: F401  (np bfloat16 support)

import concourse.bass as bass
from concourse import bacc
import concourse.mybir as mybir
from concourse.bass import ds, ts
from concourse.tile import TileContext
from concourse.bass_utils import run_bass_kernel_spmd

# ---------------------------------------------------------------- constants
B, N, C, OUT, W = 32, 4096, 256, 512, 64
EPS = 1e-5
NCORES = 8
BLOC = B // NCORES            # 4 batches per core
NTOT = BLOC * N               # 16384 points per core
KCW = C * W                   # 16384 contraction dim of final conv
NCH = N // 128                # 32 n-chunks of 128 per batch

_DT_STR = os.environ.get("KDT", "bf16")   # matmul operand dtype: bf16|f32r|f32
_DT = {
    "bf16": mybir.dt.bfloat16,
    "f32r": mybir.dt.float32r,
    "f32": mybir.dt.float32,
}[_DT_STR]
_NPDT = {"bf16": ml_dtypes.bfloat16, "f32r": np.float32, "f32": np.float32}[_DT_STR]
F32 = mybir.dt.float32
RELU = mybir.ActivationFunctionType.Relu
ALU = mybir.AluOpType


def build_bass():
    nc = bacc.Bacc("TRN2", target_bir_lowering=False, debug=True)

    # per-core inputs
    x3_d = nc.dram_tensor("x3", [3, NTOT], _DT, kind="ExternalInput")
    # featT permuted so each partition reads 1KB contiguous: [b, i, p, jj, c]
    # holds feature^T[b, n= (2*i+jj)*128 + p, c]
    ft_d = nc.dram_tensor("featT", [BLOC, N // 256, 128, 2, C], _DT,
                          kind="ExternalInput")
    w1_d = nc.dram_tensor("w1t", [3, W], _DT, kind="ExternalInput")
    w2_d = nc.dram_tensor("w2t", [128, W], _DT, kind="ExternalInput")
    w3_d = nc.dram_tensor("w3t", [128, W], _DT, kind="ExternalInput")
    b1_d = nc.dram_tensor("b1", [128, 1], F32, kind="ExternalInput")
    b2_d = nc.dram_tensor("b2", [128, 1], F32, kind="ExternalInput")
    b3_d = nc.dram_tensor("b3rep", [128, 8, W], F32, kind="ExternalInput")
    # wfT permuted so each partition reads 2KB contiguous: [i, p, jj, o]
    # holds wf'^T[cw = (2*i+jj)*128 + p, o]
    wf_d = nc.dram_tensor("wfT", [KCW // 256, 128, 2, OUT], _DT,
                          kind="ExternalInput")
    bf_d = nc.dram_tensor("bfrep", [BLOC, OUT], F32, kind="ExternalInput")
    out_d = nc.dram_tensor("out", [BLOC, OUT], F32, kind="ExternalOutput")

    with TileContext(nc) as tc:
        with (
            tc.tile_pool(name="const", bufs=1) as cpool,
            tc.tile_pool(name="hbuf", bufs=1) as hpool,
            tc.tile_pool(name="wgt", bufs=2) as wpool,
            tc.tile_pool(name="feat", bufs=6) as fpool,
            tc.tile_pool(name="wfin", bufs=22) as wfpool,
            tc.tile_pool(name="osb", bufs=1) as opool,
            tc.tile_pool(name="ph", bufs=2, space="PSUM") as pph,
            tc.tile_pool(name="pw", bufs=2, space="PSUM") as ppw,
            tc.tile_pool(name="pa", bufs=1, space="PSUM") as ppa,
            tc.tile_pool(name="pf", bufs=1, space="PSUM") as ppf,
        ):
            # constants
            x3 = cpool.tile([3, NTOT], _DT, tag="x3")
            for b in range(BLOC):
                nc.sync.dma_start(
                    out=x3[:, ds(b * N, N)], in_=x3_d[:, ds(b * N, N)]
                )
            w1t = cpool.tile([3, W], _DT, tag="w1t")
            nc.sync.dma_start(out=w1t[:], in_=w1_d[:])
            w2t = cpool.tile([128, W], _DT, tag="w2t")
            nc.sync.dma_start(out=w2t[:], in_=w2_d[:])
            w3t = cpool.tile([128, W], _DT, tag="w3t")
            nc.sync.dma_start(out=w3t[:], in_=w3_d[:])
            b1t = cpool.tile([128, 1], F32, tag="b1")
            nc.gpsimd.dma_start(out=b1t[:], in_=b1_d[:])
            b2t = cpool.tile([128, 1], F32, tag="b2")
            nc.gpsimd.dma_start(out=b2t[:], in_=b2_d[:])
            b3t = cpool.tile([128, 8, W], F32, tag="b3")
            nc.gpsimd.dma_start(out=b3t[:], in_=b3_d[:])
            bft = cpool.tile([BLOC, OUT], F32, tag="bf")
            nc.gpsimd.dma_start(out=bft[:], in_=bf_d[:])
            zeros = cpool.tile([128, 2, 512], F32, tag="zeros")
            nc.vector.memset(zeros[:], 0.0)
            ones = cpool.tile([W, 512], _DT, tag="ones")
            nc.vector.memset(ones[:], 1.0)
            # HAM primer: dense dep-free matmuls during the startup DMA window
            # keep the PE activity monitor busy so the clock un-throttles early
            prime_ps = ppf.tile([W, 512], F32, tag="fps")
            for _ in range(10):
                nc.tensor.matmul(prime_ps[:], lhsT=ones[:, 0:W], rhs=ones[:],
                                 start=True, stop=True)
            prime_sb = cpool.tile([W, 512], _DT, tag="prime")
            nc.vector.tensor_copy(prime_sb[:], prime_ps[:])
            # agg_s[p, k, b]: contraction rows cw = 128*k + p, per batch col
            agg_s = cpool.tile([128, KCW // 128, BLOC], _DT, tag="aggs")

            # wfT tiles for the final conv: prefetched throughout the kernel
            NK = KCW // 128
            wf_tiles = [None] * (NK // 4)

            def load_wf(j4):
                wt = wfpool.tile([128, 2, 2, OUT], _DT, tag="wf")
                nc.gpsimd.dma_start(
                    out=wt[:],
                    in_=wf_d[ds(2 * j4, 2)].rearrange("g p j o -> p g j o"),
                )
                wf_tiles[j4] = wt

            # h layout: partition (64*h + ch) for h in {0,1}, free (par, n);
            # batch b = 2*par + h
            h1 = hpool.tile([128, 2, N], _DT, tag="h1")
            h2 = hpool.tile([128, 2, N], _DT, tag="h2")
            # ---- layer 1: (3 -> 64), all 4 batches in one 2-bank psum
            for i in range(N // 512):
                ps = pph.tile([128, 2, 512], F32, tag="hps")
                for par in range(2):
                    for h in range(2):
                        bglob = 2 * par + h
                        nc.tensor.matmul(
                            ps[ds(64 * h, W), par, :],
                            lhsT=w1t[:],
                            rhs=x3[:, ds(bglob * N + i * 512, 512)],
                            start=True, stop=True,
                        )
                if i % 2 == 0:
                    nc.scalar.activation(
                        h1[:, :, ds(i * 512, 512)], ps[:], RELU, bias=b1t[:]
                    )
                else:
                    nc.vector.scalar_tensor_tensor(
                        h1[:, :, ds(i * 512, 512)],
                        in0=ps[:], scalar=b1t[:], in1=zeros[:],
                        op0=ALU.add, op1=ALU.max,
                    )
            # ---- layer 2: (64 -> 64)
            for i in range(N // 512):
                ps = pph.tile([128, 2, 512], F32, tag="hps")
                for par in range(2):
                    for h in range(2):
                        nc.tensor.matmul(
                            ps[ds(64 * h, W), par, :],
                            lhsT=w2t[ds(64 * h, W), :],
                            rhs=h1[ds(64 * h, W), par, ds(i * 512, 512)],
                            start=True, stop=True,
                        )
                if i % 2 == 0:
                    nc.vector.scalar_tensor_tensor(
                        h2[:, :, ds(i * 512, 512)],
                        in0=ps[:], scalar=b2t[:], in1=zeros[:],
                        op0=ALU.add, op1=ALU.max,
                    )
                else:
                    nc.scalar.activation(
                        h2[:, :, ds(i * 512, 512)], ps[:], RELU, bias=b2t[:]
                    )

            # ---- per batch: layer 3 (transposed out) software-pipelined
            # against the previous batch's bmm so PE fills feat-DMA stalls
            wgt_tiles = [None] * BLOC

            def l3_group(b, j):
                par, h = b // 2, b % 2
                if wgt_tiles[b] is None:
                    wgt_tiles[b] = wpool.tile([128, NCH, W], _DT, tag="wgt", name=f"wgt{b}")
                wgt = wgt_tiles[b]
                pwg = ppw.tile([128, 8, W], F32, tag="wps")
                for jj in range(8):
                    i = j * 8 + jj
                    nc.tensor.matmul(
                        pwg[:, jj, :],
                        lhsT=h2[ds(64 * h, W), par, ds(i * 128, 128)],
                        rhs=w3t[ds(64 * h, W), :],
                        start=True, stop=True,
                    )
                nc.vector.tensor_add(pwg[:], pwg[:], b3t[:])
                nc.scalar.activation(wgt[:, ts(j, 8), :], pwg[:], RELU)

            for j in range(NCH // 8):
                l3_group(0, j)
            for b in range(BLOC):
                wgt = wgt_tiles[b]
                # bmm: agg^T[b] (64w x 256c) += wgt_chunk^T @ feat_chunk
                pa = ppa.tile([W, C], F32, tag="aps")
                for j4 in range(NCH // 4):
                    ftile = fpool.tile([128, 2, 2, C], _DT, tag="ft")
                    nc.sync.dma_start(
                        out=ftile[:],
                        in_=ft_d[b, ds(2 * j4, 2)].rearrange("g p j c -> p g j c"),
                    )
                    for g in range(2):
                        for jj in range(2):
                            i = j4 * 4 + g * 2 + jj
                            nc.tensor.matmul(
                                pa[:],
                                lhsT=wgt[:, i, :],
                                rhs=ftile[:, g, jj, :],
                                start=(i == 0), stop=(i == NCH - 1),
                            )
                    if b < BLOC - 1 and j4 % 2 == 0:
                        l3_group(b + 1, j4 // 2)
                # shuffle agg^T (64w x 256c) -> agg_s[:, :, b] (cw-major)
                pav = pa[:].rearrange("w (k two) -> w two k", two=2)
                nc.vector.tensor_copy(agg_s[0:W, :, b], pav[:, 0, :])
                nc.vector.tensor_copy(agg_s[W:128, :, b], pav[:, 1, :])
                # interleave wf prefetch with the batch pipeline
                for j4 in range(6 * b, 6 * b + 6):
                    load_wf(j4)

            # ---- final conv: out^T (b x o) = sum_k agg_s[:,k,:].T @ wfT[k]
            # 4 concurrent accumulation chains on independent 128x32 array
            # column-tiles; chain c handles k % 4 == c, partials land on psum
            # partitions 32c..32c+3 of one bank, combined at the end.
            pf = ppf.tile([128, OUT], F32, tag="fps")
            for j4 in range(NK // 4):
                if wf_tiles[j4] is None:
                    load_wf(j4)
                wtile = wf_tiles[j4]
                for g in range(2):
                    for jj in range(2):
                        k = j4 * 4 + g * 2 + jj
                        c = k % 2
                        nc.tensor.matmul(
                            pf[ds(64 * c, BLOC), :],
                            lhsT=agg_s[:, k, :],
                            rhs=wtile[:, g, jj, :],
                            start=(j4 == 0 and g == 0), stop=(j4 == NK // 4 - 1 and g == 1),
                            tile_position=(0, 64 * c),
                            skip_group_check=True,
                        )
            part1 = opool.tile([BLOC, OUT], F32, tag="part1")
            nc.vector.tensor_copy(part1[:], pf[ds(64, BLOC), :])
            nc.vector.tensor_add(pf[ds(0, BLOC), :], pf[ds(0, BLOC), :], part1[:])
            nc.vector.tensor_add(pf[ds(0, BLOC), :], pf[ds(0, BLOC), :], bft[:])
            out_sb = opool.tile([BLOC, OUT], F32, tag="out")
            nc.scalar.activation(out_sb[:], pf[ds(0, BLOC), :], RELU)
            nc.sync.dma_start(out=out_d[:], in_=out_sb[:])

    nc.compile()
    return nc


_NC_CACHE = None


def _get_nc():
    global _NC_CACHE
    if _NC_CACHE is None:
        _NC_CACHE = build_bass()
    return _NC_CACHE


def _fold_bn(w, b, g, be, m, v):
    """Fold eval-mode BN into conv weight/bias: y = diag(s) W x + (s*(b-m)+be)."""
    s = (g / np.sqrt(v + EPS)).astype(np.float64)
    wp = (w.astype(np.float64) * s[:, None]).astype(np.float32)
    bp = (s * (b.astype(np.float64) - m) + be).astype(np.float32)
    return wp, bp


def prep_inputs(xyz, feature, w1, b1, g1, be1, m1, v1,
                w2, b2, g2, be2, m2, v2,
                w3, b3, g3, be3, m3, v3,
                wf, bf, gf, bef, mf, vf):
    """Host-side prep: BN folding, transposes, per-core sharding. Returns in_maps."""
    w1p, b1p = _fold_bn(w1, b1, g1, be1, m1, v1)
    w2p, b2p = _fold_bn(w2, b2, g2, be2, m2, v2)
    w3p, b3p = _fold_bn(w3, b3, g3, be3, m3, v3)
    wfp, bfp = _fold_bn(wf, bf, gf, bef, mf, vf)
    # fold the feat = feature/N scaling into layer-3 output (relu(x)/n == relu(x/n))
    w3p = (w3p / N).astype(np.float32)
    b3p = (b3p / N).astype(np.float32)

    # wfT permuted: [i, p, jj, o] = wf'^T[(2i+jj)*128 + p, o]
    wfT = np.ascontiguousarray(wfp.T)                       # (16384, 512)
    wfT = wfT.reshape(KCW // 256, 2, 128, OUT).transpose(0, 2, 1, 3)
    shared = {
        "w1t": np.ascontiguousarray(w1p.T).astype(_NPDT),
        "w2t": np.ascontiguousarray(np.tile(w2p.T, (2, 1))).astype(_NPDT),
        "w3t": np.ascontiguousarray(np.tile(w3p.T, (2, 1))).astype(_NPDT),
        "b1": np.tile(b1p, 2).reshape(128, 1).astype(np.float32),
        "b2": np.tile(b2p, 2).reshape(128, 1).astype(np.float32),
        "b3rep": np.tile(b3p, (128, 8, 1)).astype(np.float32),
        "wfT": np.ascontiguousarray(wfT).astype(_NPDT),
        "bfrep": np.tile(bfp, (BLOC, 1)).astype(np.float32),
    }
    in_maps = []
    for c in range(NCORES):
        xs = xyz[c * BLOC:(c + 1) * BLOC]          # (4, 4096, 3)
        x3 = np.ascontiguousarray(xs.transpose(2, 0, 1)).reshape(3, NTOT)
        fs = feature[c * BLOC:(c + 1) * BLOC]      # (4, 256, 4096)
        ftT = fs.transpose(0, 2, 1)                # (4, 4096, 256)
        # permute: [b, i, p, jj, c] = featT[b, (2i+jj)*128 + p, c]
        ftT = ftT.reshape(BLOC, N // 256, 2, 128, C).transpose(0, 1, 3, 2, 4)
        in_maps.append({
            "x3": x3.astype(_NPDT),
            "featT": np.ascontiguousarray(ftT).astype(_NPDT),
            **shared,
        })
    return in_maps


def _run(inputs, trace=False):
    inputs = {k: np.asarray(v) for k, v in inputs.items()}
    nc = _get_nc()
    in_maps = prep_inputs(
        inputs["xyz"], inputs["feature"],
        inputs["w1"], inputs["b1"], inputs["g1"], inputs["be1"], inputs["m1"], inputs["v1"],
        inputs["w2"], inputs["b2"], inputs["g2"], inputs["be2"], inputs["m2"], inputs["v2"],
        inputs["w3"], inputs["b3"], inputs["g3"], inputs["be3"], inputs["m3"], inputs["v3"],
        inputs["wf"], inputs["bf"], inputs["gf"], inputs["bef"], inputs["mf"], inputs["vf"],
    )
    res = run_bass_kernel_spmd(
        nc, in_maps, core_ids=list(range(NCORES)), trace=trace,
        trace_cores=list(range(NCORES)) if trace else None,
    )
    outs = [np.asarray(res.results[i]["out"]) for i in range(NCORES)]  # (4, 512)
    full = np.concatenate(outs, axis=0).astype(np.float32)             # (32, 512)
    return full.reshape(B, OUT, 1), res


def kernel(**inputs):
    return _run(inputs, trace=False)[0]


# revision 36
# speedup vs baseline: 1.0670x; 1.0670x over previous
"""Trainium2 Bass kernel for CNCAgg (weight-MLP + bmm aggregation + final 1x1 conv).

Strategy: data-parallel over batch B=32 across 8 NeuronCores (4 batches/core).
All BN layers are folded into the conv weights on the host (eval-mode BN is an
affine transform). Per core:
  1. WeightNet 3->64->64->64 (conv1x1+BN+ReLU x3) on the PE array, batches
     pair-stacked along the PSUM free axis for full-width activations.
  2. Layer 3 emits its output transposed (n on partitions) so the bmm
     agg^T[b] = sum_n wgt[n,:]^T feat[n,:] accumulates in PSUM with no
     on-chip transpose of `feature` (host pre-transposes feature to (B,N,C)).
  3. agg is shuffled in-SBUF into (cw, k, b) layout; the final conv runs as
     out^T[b,o] = sum_k agg_s[:,k,:].T @ wfT[k] with wf' as the wide moving
     operand (N=512) so fp32->bf16 weight streaming hits full PE rate.
"""

import os
import sys

sys.path.insert(0, "/opt/trn_rl_repo")

import numpy as np
import ml_dtypes  # noqa: F401  (np bfloat16 support)

import concourse.bass as bass
from concourse import bacc
import concourse.mybir as mybir
from concourse.bass import ds, ts
from concourse.tile import TileContext
from concourse.bass_utils import run_bass_kernel_spmd

# ---------------------------------------------------------------- constants
B, N, C, OUT, W = 32, 4096, 256, 512, 64
EPS = 1e-5
NCORES = 8
BLOC = B // NCORES            # 4 batches per core
NTOT = BLOC * N               # 16384 points per core
KCW = C * W                   # 16384 contraction dim of final conv
NCH = N // 128                # 32 n-chunks of 128 per batch

_DT_STR = os.environ.get("KDT", "bf16")   # matmul operand dtype: bf16|f32r|f32
_DT = {
    "bf16": mybir.dt.bfloat16,
    "f32r": mybir.dt.float32r,
    "f32": mybir.dt.float32,
}[_DT_STR]
_NPDT = {"bf16": ml_dtypes.bfloat16, "f32r": np.float32, "f32": np.float32}[_DT_STR]
F32 = mybir.dt.float32
RELU = mybir.ActivationFunctionType.Relu
ALU = mybir.AluOpType


def build_bass():
    nc = bacc.Bacc("TRN2", target_bir_lowering=False, debug=True)

    # per-core inputs
    x3_d = nc.dram_tensor("x3", [3, NTOT], _DT, kind="ExternalInput")
    # featT permuted so each partition reads 1KB contiguous: [b, i, p, jj, c]
    # holds feature^T[b, n= (2*i+jj)*128 + p, c]
    ft_d = nc.dram_tensor("featT", [BLOC, N // 256, 128, 2, C], _DT,
                          kind="ExternalInput")
    w1_d = nc.dram_tensor("w1t", [3, W], _DT, kind="ExternalInput")
    w2_d = nc.dram_tensor("w2t", [128, W], _DT, kind="ExternalInput")
    w3_d = nc.dram_tensor("w3t", [128, W], _DT, kind="ExternalInput")
    b1_d = nc.dram_tensor("b1", [128, 1], F32, kind="ExternalInput")
    b2_d = nc.dram_tensor("b2", [128, 1], F32, kind="ExternalInput")
    b3_d = nc.dram_tensor("b3rep", [128, 8, W], F32, kind="ExternalInput")
    # wfT permuted so each partition reads 2KB contiguous: [i, p, jj, o]
    # holds wf'^T[cw = (2*i+jj)*128 + p, o]
    wf_d = nc.dram_tensor("wfT", [KCW // 256, 128, 2, OUT], _DT,
                          kind="ExternalInput")
    bf_d = nc.dram_tensor("bfrep", [BLOC, OUT], F32, kind="ExternalInput")
    out_d = nc.dram_tensor("out", [BLOC, OUT], F32, kind="ExternalOutput")

    with TileContext(nc) as tc:
        with (
            tc.tile_pool(name="const", bufs=1) as cpool,
            tc.tile_pool(name="hbuf", bufs=1) as hpool,
            tc.tile_pool(name="wgt", bufs=2) as wpool,
            tc.tile_pool(name="feat", bufs=10) as fpool,
            tc.tile_pool(name="wfin", bufs=21) as wfpool,
            tc.tile_pool(name="osb", bufs=1) as opool,
            tc.tile_pool(name="ph", bufs=2, space="PSUM") as pph,
            tc.tile_pool(name="pw", bufs=2, space="PSUM") as ppw,
            tc.tile_pool(name="pa", bufs=1, space="PSUM") as ppa,
            tc.tile_pool(name="pf", bufs=1, space="PSUM") as ppf,
        ):
            # constants (small weights first so L1 can start immediately)
            w1t = cpool.tile([3, W], _DT, tag="w1t")
            nc.sync.dma_start(out=w1t[:], in_=w1_d[:])
            w2t = cpool.tile([128, W], _DT, tag="w2t")
            nc.sync.dma_start(out=w2t[:], in_=w2_d[:])
            w3t = cpool.tile([128, W], _DT, tag="w3t")
            nc.sync.dma_start(out=w3t[:], in_=w3_d[:])
            x3 = cpool.tile([3, NTOT], _DT, tag="x3")
            for b in range(BLOC):
                nc.sync.dma_start(
                    out=x3[:, ds(b * N, N)], in_=x3_d[:, ds(b * N, N)]
                )
            b1t = cpool.tile([128, 1], F32, tag="b1")
            nc.gpsimd.dma_start(out=b1t[:], in_=b1_d[:])
            b2t = cpool.tile([128, 1], F32, tag="b2")
            nc.gpsimd.dma_start(out=b2t[:], in_=b2_d[:])
            b3t = cpool.tile([128, 8, W], F32, tag="b3")
            nc.gpsimd.dma_start(out=b3t[:], in_=b3_d[:])
            bft = cpool.tile([BLOC, OUT], F32, tag="bf")
            nc.gpsimd.dma_start(out=bft[:], in_=bf_d[:])
            zeros = cpool.tile([128, 2, 512], F32, tag="zeros")
            nc.vector.memset(zeros[:], 0.0)
            ones = cpool.tile([W, 512], _DT, tag="ones")
            nc.vector.memset(ones[:], 1.0)
            # HAM primer: dense dep-free matmuls during the startup DMA window
            # keep the PE activity monitor busy so the clock un-throttles early
            prime_ps = ppf.tile([W, 512], F32, tag="fps")
            for _ in range(10):
                nc.tensor.matmul(prime_ps[:], lhsT=ones[:, 0:W], rhs=ones[:],
                                 start=True, stop=True)
            prime_sb = cpool.tile([W, 512], _DT, tag="prime")
            nc.vector.tensor_copy(prime_sb[:], prime_ps[:])
            # agg_s[p, k, b]: contraction rows cw = 128*k + p, per batch col
            agg_s = cpool.tile([128, KCW // 128, BLOC], _DT, tag="aggs")

            # feat tiles, preloadable ahead of each batch's bmm
            ft_tiles = {}

            def load_ft(b, j4):
                ft = fpool.tile([128, 2, 2, C], _DT, tag="ft",
                                name=f"ft{b}_{j4}")
                nc.sync.dma_start(
                    out=ft[:],
                    in_=ft_d[b, ds(2 * j4, 2)].rearrange("g p j c -> p g j c"),
                )
                ft_tiles[(b, j4)] = ft

            # prefetch the first batches' feature tiles during L1/L2
            for j4 in range(NCH // 4):
                load_ft(0, j4)
            for j4 in range(2):
                load_ft(1, j4)

            # wfT tiles for the final conv: prefetched throughout the kernel
            NK = KCW // 128
            wf_tiles = [None] * (NK // 4)

            def load_wf(j4):
                wt = wfpool.tile([128, 2, 2, OUT], _DT, tag="wf")
                nc.gpsimd.dma_start(
                    out=wt[:],
                    in_=wf_d[ds(2 * j4, 2)].rearrange("g p j o -> p g j o"),
                )
                wf_tiles[j4] = wt

            # h layout: partition (64*h + ch) for h in {0,1}, free (par, n);
            # batch b = 2*par + h
            h1 = hpool.tile([128, 2, N], _DT, tag="h1")
            h2 = hpool.tile([128, 2, N], _DT, tag="h2")
            # ---- layer 1: (3 -> 64), all 4 batches in one 2-bank psum
            for i in range(N // 512):
                ps = pph.tile([128, 2, 512], F32, tag="hps")
                for par in range(2):
                    for h in range(2):
                        bglob = 2 * par + h
                        nc.tensor.matmul(
                            ps[ds(64 * h, W), par, :],
                            lhsT=w1t[:],
                            rhs=x3[:, ds(bglob * N + i * 512, 512)],
                            start=True, stop=True,
                        )
                if i % 2 == 0:
                    nc.scalar.activation(
                        h1[:, :, ds(i * 512, 512)], ps[:], RELU, bias=b1t[:]
                    )
                else:
                    nc.vector.scalar_tensor_tensor(
                        h1[:, :, ds(i * 512, 512)],
                        in0=ps[:], scalar=b1t[:], in1=zeros[:],
                        op0=ALU.add, op1=ALU.max,
                    )
            # ---- layer 2: (64 -> 64)
            for i in range(N // 512):
                ps = pph.tile([128, 2, 512], F32, tag="hps")
                for par in range(2):
                    for h in range(2):
                        nc.tensor.matmul(
                            ps[ds(64 * h, W), par, :],
                            lhsT=w2t[ds(64 * h, W), :],
                            rhs=h1[ds(64 * h, W), par, ds(i * 512, 512)],
                            start=True, stop=True,
                        )
                if i % 2 == 0:
                    nc.vector.scalar_tensor_tensor(
                        h2[:, :, ds(i * 512, 512)],
                        in0=ps[:], scalar=b2t[:], in1=zeros[:],
                        op0=ALU.add, op1=ALU.max,
                    )
                else:
                    nc.scalar.activation(
                        h2[:, :, ds(i * 512, 512)], ps[:], RELU, bias=b2t[:]
                    )

            # ---- per batch: layer 3 (transposed out) software-pipelined
            # against the previous batch's bmm so PE fills feat-DMA stalls
            wgt_tiles = [None] * BLOC

            def l3_group(b, j):
                par, h = b // 2, b % 2
                if wgt_tiles[b] is None:
                    wgt_tiles[b] = wpool.tile([128, NCH, W], _DT, tag="wgt", name=f"wgt{b}")
                wgt = wgt_tiles[b]
                pwg = ppw.tile([128, 8, W], F32, tag="wps")
                for jj in range(8):
                    i = j * 8 + jj
                    nc.tensor.matmul(
                        pwg[:, jj, :],
                        lhsT=h2[ds(64 * h, W), par, ds(i * 128, 128)],
                        rhs=w3t[ds(64 * h, W), :],
                        start=True, stop=True,
                    )
                nc.vector.tensor_add(pwg[:], pwg[:], b3t[:])
                nc.scalar.activation(wgt[:, ts(j, 8), :], pwg[:], RELU)

            for j in range(NCH // 8):
                l3_group(0, j)
            for b in range(BLOC):
                wgt = wgt_tiles[b]
                # bmm: agg^T[b] (64w x 256c) += wgt_chunk^T @ feat_chunk
                pa = ppa.tile([W, C], F32, tag="aps")
                for j4 in range(NCH // 4):
                    if (b, j4) not in ft_tiles:
                        load_ft(b, j4)
                    ftile = ft_tiles[(b, j4)]
                    for g in range(2):
                        for jj in range(2):
                            i = j4 * 4 + g * 2 + jj
                            nc.tensor.matmul(
                                pa[:],
                                lhsT=wgt[:, i, :],
                                rhs=ftile[:, g, jj, :],
                                start=(i == 0), stop=(i == NCH - 1),
                            )
                    if b < BLOC - 1 and j4 % 2 == 0:
                        l3_group(b + 1, j4 // 2)
                # shuffle agg^T (64w x 256c) -> agg_s[:, :, b] (cw-major)
                pav = pa[:].rearrange("w (k two) -> w two k", two=2)
                nc.vector.tensor_copy(agg_s[0:W, :, b], pav[:, 0, :])
                nc.vector.tensor_copy(agg_s[W:128, :, b], pav[:, 1, :])
                # interleave wf prefetch with the batch pipeline
                for j4 in range(6 * b, 6 * b + 6):
                    load_wf(j4)

            # ---- final conv: out^T (b x o) = sum_k agg_s[:,k,:].T @ wfT[k]
            # 4 concurrent accumulation chains on independent 128x32 array
            # column-tiles; chain c handles k % 4 == c, partials land on psum
            # partitions 32c..32c+3 of one bank, combined at the end.
            pf = ppf.tile([128, OUT], F32, tag="fps")
            for j4 in range(NK // 4):
                if wf_tiles[j4] is None:
                    load_wf(j4)
                wtile = wf_tiles[j4]
                for g in range(2):
                    for jj in range(2):
                        k = j4 * 4 + g * 2 + jj
                        c = k % 2
                        nc.tensor.matmul(
                            pf[ds(64 * c, BLOC), :],
                            lhsT=agg_s[:, k, :],
                            rhs=wtile[:, g, jj, :],
                            start=(j4 == 0 and g == 0), stop=(j4 == NK // 4 - 1 and g == 1),
                            tile_position=(0, 64 * c),
                            skip_group_check=True,
                        )
            part1 = opool.tile([BLOC, OUT], F32, tag="part1")
            nc.vector.tensor_copy(part1[:], pf[ds(64, BLOC), :])
            nc.vector.tensor_add(pf[ds(0, BLOC), :], pf[ds(0, BLOC), :], part1[:])
            nc.vector.tensor_add(pf[ds(0, BLOC), :], pf[ds(0, BLOC), :], bft[:])
            out_sb = opool.tile([BLOC, OUT], F32, tag="out")
            nc.scalar.activation(out_sb[:], pf[ds(0, BLOC), :], RELU)
            nc.sync.dma_start(out=out_d[:], in_=out_sb[:])

    nc.compile()
    return nc


_NC_CACHE = None


def _get_nc():
    global _NC_CACHE
    if _NC_CACHE is None:
        _NC_CACHE = build_bass()
    return _NC_CACHE


def _fold_bn(w, b, g, be, m, v):
    """Fold eval-mode BN into conv weight/bias: y = diag(s) W x + (s*(b-m)+be)."""
    s = (g / np.sqrt(v + EPS)).astype(np.float64)
    wp = (w.astype(np.float64) * s[:, None]).astype(np.float32)
    bp = (s * (b.astype(np.float64) - m) + be).astype(np.float32)
    return wp, bp


def prep_inputs(xyz, feature, w1, b1, g1, be1, m1, v1,
                w2, b2, g2, be2, m2, v2,
                w3, b3, g3, be3, m3, v3,
                wf, bf, gf, bef, mf, vf):
    """Host-side prep: BN folding, transposes, per-core sharding. Returns in_maps."""
    w1p, b1p = _fold_bn(w1, b1, g1, be1, m1, v1)
    w2p, b2p = _fold_bn(w2, b2, g2, be2, m2, v2)
    w3p, b3p = _fold_bn(w3, b3, g3, be3, m3, v3)
    wfp, bfp = _fold_bn(wf, bf, gf, bef, mf, vf)
    # fold the feat = feature/N scaling into layer-3 output (relu(x)/n == relu(x/n))
    w3p = (w3p / N).astype(np.float32)
    b3p = (b3p / N).astype(np.float32)

    # wfT permuted: [i, p, jj, o] = wf'^T[(2i+jj)*128 + p, o]
    wfT = np.ascontiguousarray(wfp.T)                       # (16384, 512)
    wfT = wfT.reshape(KCW // 256, 2, 128, OUT).transpose(0, 2, 1, 3)
    shared = {
        "w1t": np.ascontiguousarray(w1p.T).astype(_NPDT),
        "w2t": np.ascontiguousarray(np.tile(w2p.T, (2, 1))).astype(_NPDT),
        "w3t": np.ascontiguousarray(np.tile(w3p.T, (2, 1))).astype(_NPDT),
        "b1": np.tile(b1p, 2).reshape(128, 1).astype(np.float32),
        "b2": np.tile(b2p, 2).reshape(128, 1).astype(np.float32),
        "b3rep": np.tile(b3p, (128, 8, 1)).astype(np.float32),
        "wfT": np.ascontiguousarray(wfT).astype(_NPDT),
        "bfrep": np.tile(bfp, (BLOC, 1)).astype(np.float32),
    }
    in_maps = []
    for c in range(NCORES):
        xs = xyz[c * BLOC:(c + 1) * BLOC]          # (4, 4096, 3)
        x3 = np.ascontiguousarray(xs.transpose(2, 0, 1)).reshape(3, NTOT)
        fs = feature[c * BLOC:(c + 1) * BLOC]      # (4, 256, 4096)
        ftT = fs.transpose(0, 2, 1)                # (4, 4096, 256)
        # permute: [b, i, p, jj, c] = featT[b, (2i+jj)*128 + p, c]
        ftT = ftT.reshape(BLOC, N // 256, 2, 128, C).transpose(0, 1, 3, 2, 4)
        in_maps.append({
            "x3": x3.astype(_NPDT),
            "featT": np.ascontiguousarray(ftT).astype(_NPDT),
            **shared,
        })
    return in_maps


def _run(inputs, trace=False):
    inputs = {k: np.asarray(v) for k, v in inputs.items()}
    nc = _get_nc()
    in_maps = prep_inputs(
        inputs["xyz"], inputs["feature"],
        inputs["w1"], inputs["b1"], inputs["g1"], inputs["be1"], inputs["m1"], inputs["v1"],
        inputs["w2"], inputs["b2"], inputs["g2"], inputs["be2"], inputs["m2"], inputs["v2"],
        inputs["w3"], inputs["b3"], inputs["g3"], inputs["be3"], inputs["m3"], inputs["v3"],
        inputs["wf"], inputs["bf"], inputs["gf"], inputs["bef"], inputs["mf"], inputs["vf"],
    )
    res = run_bass_kernel_spmd(
        nc, in_maps, core_ids=list(range(NCORES)), trace=trace,
        trace_cores=list(range(NCORES)) if trace else None,
    )
    outs = [np.asarray(res.results[i]["out"]) for i in range(NCORES)]  # (4, 512)
    full = np.concatenate(outs, axis=0).astype(np.float32)             # (32, 512)
    return full.reshape(B, OUT, 1), res


def kernel(**inputs):
    return _run(inputs, trace=False)[0]


# revision 38
# speedup vs baseline: 1.0933x; 1.0246x over previous
"""Trainium2 Bass kernel for CNCAgg (weight-MLP + bmm aggregation + final 1x1 conv).

Strategy: data-parallel over batch B=32 across 8 NeuronCores (4 batches/core).
All BN layers are folded into the conv weights on the host (eval-mode BN is an
affine transform). Per core:
  1. WeightNet 3->64->64->64 (conv1x1+BN+ReLU x3) on the PE array, batches
     pair-stacked along the PSUM free axis for full-width activations.
  2. Layer 3 emits its output transposed (n on partitions) so the bmm
     agg^T[b] = sum_n wgt[n,:]^T feat[n,:] accumulates in PSUM with no
     on-chip transpose of `feature` (host pre-transposes feature to (B,N,C)).
  3. agg is shuffled in-SBUF into (cw, k, b) layout; the final conv runs as
     out^T[b,o] = sum_k agg_s[:,k,:].T @ wfT[k] with wf' as the wide moving
     operand (N=512) so fp32->bf16 weight streaming hits full PE rate.
"""

import os
import sys

sys.path.insert(0, "/opt/trn_rl_repo")

import numpy as np
import ml_dtypes  # noqa: F401  (np bfloat16 support)

import concourse.bass as bass
from concourse import bacc
import concourse.mybir as mybir
from concourse.bass import ds, ts
from concourse.tile import TileContext
from concourse.bass_utils import run_bass_kernel_spmd

# ---------------------------------------------------------------- constants
B, N, C, OUT, W = 32, 4096, 256, 512, 64
EPS = 1e-5
NCORES = 8
BLOC = B // NCORES            # 4 batches per core
NTOT = BLOC * N               # 16384 points per core
KCW = C * W                   # 16384 contraction dim of final conv
NCH = N // 128                # 32 n-chunks of 128 per batch

_DT_STR = os.environ.get("KDT", "bf16")   # matmul operand dtype: bf16|f32r|f32
_DT = {
    "bf16": mybir.dt.bfloat16,
    "f32r": mybir.dt.float32r,
    "f32": mybir.dt.float32,
}[_DT_STR]
_NPDT = {"bf16": ml_dtypes.bfloat16, "f32r": np.float32, "f32": np.float32}[_DT_STR]
F32 = mybir.dt.float32
RELU = mybir.ActivationFunctionType.Relu
ALU = mybir.AluOpType


def build_bass():
    nc = bacc.Bacc("TRN2", target_bir_lowering=False, debug=True)

    # per-core inputs
    x3_d = nc.dram_tensor("x3", [3, NTOT], _DT, kind="ExternalInput")
    # featT permuted so each partition reads 1KB contiguous: [b, i, p, jj, c]
    # holds feature^T[b, n= (2*i+jj)*128 + p, c]
    ft_d = nc.dram_tensor("featT", [BLOC, N // 256, 128, 2, C], _DT,
                          kind="ExternalInput")
    w1_d = nc.dram_tensor("w1t", [3, W], _DT, kind="ExternalInput")
    w2_d = nc.dram_tensor("w2t", [128, W], _DT, kind="ExternalInput")
    w3_d = nc.dram_tensor("w3t", [128, W], _DT, kind="ExternalInput")
    b1_d = nc.dram_tensor("b1", [128, 1], F32, kind="ExternalInput")
    b2_d = nc.dram_tensor("b2", [128, 1], F32, kind="ExternalInput")
    b3_d = nc.dram_tensor("b3rep", [128, 8, W], F32, kind="ExternalInput")
    # wfT permuted so each partition reads 2KB contiguous: [i, p, jj, o]
    # holds wf'^T[cw = (2*i+jj)*128 + p, o]
    wf_d = nc.dram_tensor("wfT", [KCW // 256, 128, 2, OUT], _DT,
                          kind="ExternalInput")
    bf_d = nc.dram_tensor("bfrep", [BLOC, OUT], F32, kind="ExternalInput")
    out_d = nc.dram_tensor("out", [BLOC, OUT], F32, kind="ExternalOutput")

    with TileContext(nc) as tc:
        with (
            tc.tile_pool(name="const", bufs=1) as cpool,
            tc.tile_pool(name="hbuf", bufs=1) as hpool,
            tc.tile_pool(name="wgt", bufs=2) as wpool,
            tc.tile_pool(name="feat", bufs=10) as fpool,
            tc.tile_pool(name="wfin", bufs=21) as wfpool,
            tc.tile_pool(name="osb", bufs=1) as opool,
            tc.tile_pool(name="ph", bufs=2, space="PSUM") as pph,
            tc.tile_pool(name="pw", bufs=2, space="PSUM") as ppw,
            tc.tile_pool(name="pa", bufs=1, space="PSUM") as ppa,
            tc.tile_pool(name="pf", bufs=1, space="PSUM") as ppf,
        ):
            # constants (small weights first so L1 can start immediately)
            w1t = cpool.tile([3, W], _DT, tag="w1t")
            nc.sync.dma_start(out=w1t[:], in_=w1_d[:])
            w2t = cpool.tile([128, W], _DT, tag="w2t")
            nc.sync.dma_start(out=w2t[:], in_=w2_d[:])
            w3t = cpool.tile([128, W], _DT, tag="w3t")
            nc.sync.dma_start(out=w3t[:], in_=w3_d[:])
            x3 = cpool.tile([3, NTOT], _DT, tag="x3")
            for b in range(BLOC):
                nc.sync.dma_start(
                    out=x3[:, ds(b * N, N)], in_=x3_d[:, ds(b * N, N)]
                )
            b1t = cpool.tile([128, 1], F32, tag="b1")
            nc.gpsimd.dma_start(out=b1t[:], in_=b1_d[:])
            b2t = cpool.tile([128, 1], F32, tag="b2")
            nc.gpsimd.dma_start(out=b2t[:], in_=b2_d[:])
            b3t = cpool.tile([128, 8, W], F32, tag="b3")
            nc.gpsimd.dma_start(out=b3t[:], in_=b3_d[:])
            bft = cpool.tile([BLOC, OUT], F32, tag="bf")
            nc.gpsimd.dma_start(out=bft[:], in_=bf_d[:])
            zeros = cpool.tile([128, 2, 512], F32, tag="zeros")
            nc.vector.memset(zeros[:], 0.0)
            ones = cpool.tile([W, 512], _DT, tag="ones")
            nc.vector.memset(ones[:], 1.0)
            # HAM primer: dense dep-free matmuls during the startup DMA window
            # keep the PE activity monitor busy so the clock un-throttles early
            prime_ps = ppf.tile([W, 512], F32, tag="fps")
            for _ in range(10):
                nc.tensor.matmul(prime_ps[:], lhsT=ones[:, 0:W], rhs=ones[:],
                                 start=True, stop=True)
            prime_sb = cpool.tile([W, 512], _DT, tag="prime")
            nc.vector.tensor_copy(prime_sb[:], prime_ps[:])
            # agg_s[p, k, b]: contraction rows cw = 128*k + p, per batch col
            agg_s = cpool.tile([128, KCW // 128, BLOC], _DT, tag="aggs")

            # feat tiles, preloadable ahead of each batch's bmm
            ft_tiles = {}

            def load_ft(b, j4):
                ft = fpool.tile([128, 2, 2, C], _DT, tag="ft",
                                name=f"ft{b}_{j4}")
                nc.sync.dma_start(
                    out=ft[:],
                    in_=ft_d[b, ds(2 * j4, 2)].rearrange("g p j c -> p g j c"),
                )
                ft_tiles[(b, j4)] = ft

            # prefetch the first batches' feature tiles during L1/L2
            for j4 in range(NCH // 4):
                load_ft(0, j4)
            for j4 in range(2):
                load_ft(1, j4)

            # wfT tiles for the final conv: prefetched throughout the kernel
            NK = KCW // 128
            wf_tiles = [None] * (NK // 4)

            def load_wf(j4):
                wt = wfpool.tile([128, 2, 2, OUT], _DT, tag="wf",
                                 name=f"wf{j4}")
                eng = (nc.gpsimd, nc.sync)[j4 % 2]
                eng.dma_start(
                    out=wt[:],
                    in_=wf_d[ds(2 * j4, 2)].rearrange("g p j o -> p g j o"),
                )
                wf_tiles[j4] = wt

            # h layout: partition (64*h + ch) for h in {0,1}, free (par, n);
            # batch b = 2*par + h
            h1 = hpool.tile([128, 2, N], _DT, tag="h1")
            h2 = hpool.tile([128, 2, N], _DT, tag="h2")
            # ---- layer 1: (3 -> 64), all 4 batches in one 2-bank psum
            for i in range(N // 512):
                ps = pph.tile([128, 2, 512], F32, tag="hps")
                for par in range(2):
                    for h in range(2):
                        bglob = 2 * par + h
                        nc.tensor.matmul(
                            ps[ds(64 * h, W), par, :],
                            lhsT=w1t[:],
                            rhs=x3[:, ds(bglob * N + i * 512, 512)],
                            start=True, stop=True,
                        )
                if i % 2 == 0:
                    nc.scalar.activation(
                        h1[:, :, ds(i * 512, 512)], ps[:], RELU, bias=b1t[:]
                    )
                else:
                    nc.vector.scalar_tensor_tensor(
                        h1[:, :, ds(i * 512, 512)],
                        in0=ps[:], scalar=b1t[:], in1=zeros[:],
                        op0=ALU.add, op1=ALU.max,
                    )
            # ---- layer 2: (64 -> 64)
            for i in range(N // 512):
                ps = pph.tile([128, 2, 512], F32, tag="hps")
                for par in range(2):
                    for h in range(2):
                        nc.tensor.matmul(
                            ps[ds(64 * h, W), par, :],
                            lhsT=w2t[ds(64 * h, W), :],
                            rhs=h1[ds(64 * h, W), par, ds(i * 512, 512)],
                            start=True, stop=True,
                        )
                if i % 2 == 0:
                    nc.vector.scalar_tensor_tensor(
                        h2[:, :, ds(i * 512, 512)],
                        in0=ps[:], scalar=b2t[:], in1=zeros[:],
                        op0=ALU.add, op1=ALU.max,
                    )
                else:
                    nc.scalar.activation(
                        h2[:, :, ds(i * 512, 512)], ps[:], RELU, bias=b2t[:]
                    )

            # ---- per batch: layer 3 (transposed out) software-pipelined
            # against the previous batch's bmm so PE fills feat-DMA stalls
            wgt_tiles = [None] * BLOC

            def l3_group(b, j):
                par, h = b // 2, b % 2
                if wgt_tiles[b] is None:
                    wgt_tiles[b] = wpool.tile([128, NCH, W], _DT, tag="wgt", name=f"wgt{b}")
                wgt = wgt_tiles[b]
                pwg = ppw.tile([128, 8, W], F32, tag="wps")
                for jj in range(8):
                    i = j * 8 + jj
                    nc.tensor.matmul(
                        pwg[:, jj, :],
                        lhsT=h2[ds(64 * h, W), par, ds(i * 128, 128)],
                        rhs=w3t[ds(64 * h, W), :],
                        start=True, stop=True,
                    )
                nc.vector.tensor_add(pwg[:], pwg[:], b3t[:])
                nc.scalar.activation(wgt[:, ts(j, 8), :], pwg[:], RELU)

            for j in range(NCH // 8):
                l3_group(0, j)
            for b in range(BLOC):
                wgt = wgt_tiles[b]
                # bmm: agg^T[b] (64w x 256c) += wgt_chunk^T @ feat_chunk
                pa = ppa.tile([W, C], F32, tag="aps")
                for j4 in range(NCH // 4):
                    if (b, j4) not in ft_tiles:
                        load_ft(b, j4)
                    ftile = ft_tiles[(b, j4)]
                    for g in range(2):
                        for jj in range(2):
                            i = j4 * 4 + g * 2 + jj
                            nc.tensor.matmul(
                                pa[:],
                                lhsT=wgt[:, i, :],
                                rhs=ftile[:, g, jj, :],
                                start=(i == 0), stop=(i == NCH - 1),
                            )
                    if b < BLOC - 1 and j4 % 2 == 0:
                        l3_group(b + 1, j4 // 2)
                # shuffle agg^T (64w x 256c) -> agg_s[:, :, b] (cw-major)
                pav = pa[:].rearrange("w (k two) -> w two k", two=2)
                nc.vector.tensor_copy(agg_s[0:W, :, b], pav[:, 0, :])
                nc.vector.tensor_copy(agg_s[W:128, :, b], pav[:, 1, :])
                # interleave wf prefetch with the batch pipeline
                for j4 in range(6 * b, 6 * b + 6):
                    load_wf(j4)

            # ---- final conv: out^T (b x o) = sum_k agg_s[:,k,:].T @ wfT[k]
            # 4 concurrent accumulation chains on independent 128x32 array
            # column-tiles; chain c handles k % 4 == c, partials land on psum
            # partitions 32c..32c+3 of one bank, combined at the end.
            pf = ppf.tile([128, OUT], F32, tag="fps")
            for j4 in range(NK // 4):
                if wf_tiles[j4] is None:
                    load_wf(j4)
            for j4 in range(NK // 4):
                wtile = wf_tiles[j4]
                for g in range(2):
                    for jj in range(2):
                        k = j4 * 4 + g * 2 + jj
                        c = k % 2
                        nc.tensor.matmul(
                            pf[ds(64 * c, BLOC), :],
                            lhsT=agg_s[:, k, :],
                            rhs=wtile[:, g, jj, :],
                            start=(j4 == 0 and g == 0), stop=(j4 == NK // 4 - 1 and g == 1),
                            tile_position=(0, 64 * c),
                            skip_group_check=True,
                        )
            part1 = opool.tile([BLOC, OUT], F32, tag="part1")
            nc.vector.tensor_copy(part1[:], pf[ds(64, BLOC), :])
            nc.vector.tensor_add(pf[ds(0, BLOC), :], pf[ds(0, BLOC), :], part1[:])
            nc.vector.tensor_add(pf[ds(0, BLOC), :], pf[ds(0, BLOC), :], bft[:])
            out_sb = opool.tile([BLOC, OUT], F32, tag="out")
            nc.scalar.activation(out_sb[:], pf[ds(0, BLOC), :], RELU)
            nc.sync.dma_start(out=out_d[:], in_=out_sb[:])

    nc.compile()
    return nc


_NC_CACHE = None


def _get_nc():
    global _NC_CACHE
    if _NC_CACHE is None:
        _NC_CACHE = build_bass()
    return _NC_CACHE


def _fold_bn(w, b, g, be, m, v):
    """Fold eval-mode BN into conv weight/bias: y = diag(s) W x + (s*(b-m)+be)."""
    s = (g / np.sqrt(v + EPS)).astype(np.float64)
    wp = (w.astype(np.float64) * s[:, None]).astype(np.float32)
    bp = (s * (b.astype(np.float64) - m) + be).astype(np.float32)
    return wp, bp


def prep_inputs(xyz, feature, w1, b1, g1, be1, m1, v1,
                w2, b2, g2, be2, m2, v2,
                w3, b3, g3, be3, m3, v3,
                wf, bf, gf, bef, mf, vf):
    """Host-side prep: BN folding, transposes, per-core sharding. Returns in_maps."""
    w1p, b1p = _fold_bn(w1, b1, g1, be1, m1, v1)
    w2p, b2p = _fold_bn(w2, b2, g2, be2, m2, v2)
    w3p, b3p = _fold_bn(w3, b3, g3, be3, m3, v3)
    wfp, bfp = _fold_bn(wf, bf, gf, bef, mf, vf)
    # fold the feat = feature/N scaling into layer-3 output (relu(x)/n == relu(x/n))
    w3p = (w3p / N).astype(np.float32)
    b3p = (b3p / N).astype(np.float32)

    # wfT permuted: [i, p, jj, o] = wf'^T[(2i+jj)*128 + p, o]
    wfT = np.ascontiguousarray(wfp.T)                       # (16384, 512)
    wfT = wfT.reshape(KCW // 256, 2, 128, OUT).transpose(0, 2, 1, 3)
    shared = {
        "w1t": np.ascontiguousarray(w1p.T).astype(_NPDT),
        "w2t": np.ascontiguousarray(np.tile(w2p.T, (2, 1))).astype(_NPDT),
        "w3t": np.ascontiguousarray(np.tile(w3p.T, (2, 1))).astype(_NPDT),
        "b1": np.tile(b1p, 2).reshape(128, 1).astype(np.float32),
        "b2": np.tile(b2p, 2).reshape(128, 1).astype(np.float32),
        "b3rep": np.tile(b3p, (128, 8, 1)).astype(np.float32),
        "wfT": np.ascontiguousarray(wfT).astype(_NPDT),
        "bfrep": np.tile(bfp, (BLOC, 1)).astype(np.float32),
    }
    in_maps = []
    for c in range(NCORES):
        xs = xyz[c * BLOC:(c + 1) * BLOC]          # (4, 4096, 3)
        x3 = np.ascontiguousarray(xs.transpose(2, 0, 1)).reshape(3, NTOT)
        fs = feature[c * BLOC:(c + 1) * BLOC]      # (4, 256, 4096)
        ftT = fs.transpose(0, 2, 1)                # (4, 4096, 256)
        # permute: [b, i, p, jj, c] = featT[b, (2i+jj)*128 + p, c]
        ftT = ftT.reshape(BLOC, N // 256, 2, 128, C).transpose(0, 1, 3, 2, 4)
        in_maps.append({
            "x3": x3.astype(_NPDT),
            "featT": np.ascontiguousarray(ftT).astype(_NPDT),
            **shared,
        })
    return in_maps


def _run(inputs, trace=False):
    inputs = {k: np.asarray(v) for k, v in inputs.items()}
    nc = _get_nc()
    in_maps = prep_inputs(
        inputs["xyz"], inputs["feature"],
        inputs["w1"], inputs["b1"], inputs["g1"], inputs["be1"], inputs["m1"], inputs["v1"],
        inputs["w2"], inputs["b2"], inputs["g2"], inputs["be2"], inputs["m2"], inputs["v2"],
        inputs["w3"], inputs["b3"], inputs["g3"], inputs["be3"], inputs["m3"], inputs["v3"],
        inputs["wf"], inputs["bf"], inputs["gf"], inputs["bef"], inputs["mf"], inputs["vf"],
    )
    res = run_bass_kernel_spmd(
        nc, in_maps, core_ids=list(range(NCORES)), trace=trace,
        trace_cores=list(range(NCORES)) if trace else None,
    )
    outs = [np.asarray(res.results[i]["out"]) for i in range(NCORES)]  # (4, 512)
    full = np.concatenate(outs, axis=0).astype(np.float32)             # (32, 512)
    return full.reshape(B, OUT, 1), res


def kernel(**inputs):
    return _run(inputs, trace=False)[0]
